# revision 4
# baseline (speedup 1.0000x reference)
"""Trainium2 Bass kernel for ContextHyperMatrix (MoE-style routed vec-mat).

Reference computation:
    w = weight[context[:, 0]]              # [B, IN, OUT] gather
    out = einsum('bx,bxy->by', x, w)       # [B, OUT]

Shapes: x [32768, 128] f32, weight [1024, 128, 128] f32, context [32768, 1] i64.

Strategy (expert-parallel, fully static device program):
  - Core c owns experts [c*EPC, (c+1)*EPC), EPC = 1024/8 = 128. Each core
    streams its own contiguous 8 MB weight slab with plain sequential DMAs —
    no indirect/dynamic addressing on device.
  - The host routes samples: sort by expert, transpose, and pad every
    expert's sample group to a uniform K columns, so every core runs the
    exact same static instruction stream (SPMD) on different data.
  - Device, per expert j: load W[j] [IN=128, OUT=128] to SBUF, matmul
    psum[OUT, K] = W[j].T-free x.T columns (lhsT=W so psum = out.T),
    copy psum -> SBUF, DMA out.T columns back.
  - Host scatters out.T columns back to the original sample order.
"""

import os

import numpy as np

# Populated by kernel() after each run; test harness reads timing from here.
LAST_RESULT = None
LAST_NC = None

_CORES = 8


def _build_program(IN, OUT, EPC, K, GRP):
    import concourse.mybir as mybir
    import concourse.tile as tile
    from concourse import bacc

    nc = bacc.Bacc(
        "TRN2",
        target_bir_lowering=False,
        debug=False,
        num_devices=_CORES,
    )
    NCOL = EPC * K
    dt = mybir.dt.float32
    xt_d = nc.dram_tensor("xt", [IN, NCOL], dt, kind="ExternalInput").ap()
    w_d = nc.dram_tensor("w", [EPC, IN, OUT], dt, kind="ExternalInput").ap()
    out_d = nc.dram_tensor("outt", [OUT, NCOL], dt, kind="ExternalOutput").ap()

    GCOL = GRP * K  # columns per x/out DMA group
    with tile.TileContext(nc) as tc:
        with (
            tc.tile_pool(name="xbuf", bufs=4) as xpool,
            tc.tile_pool(name="obuf", bufs=4) as opool,
            tc.tile_pool(name="wbuf", bufs=6) as wpool,
            tc.tile_pool(name="psum", bufs=8, space="PSUM") as ppool,
        ):
            for g in range(EPC // GRP):
                x_t = xpool.tile([IN, GCOL], dt)
                nc.sync.dma_start(
                    out=x_t[:], in_=xt_d[:, g * GCOL : (g + 1) * GCOL]
                )
                o_t = opool.tile([OUT, GCOL], dt)
                for jj in range(GRP):
                    j = g * GRP + jj
                    w_t = wpool.tile([IN, OUT], dt)
                    nc.sync.dma_start(out=w_t[:], in_=w_d[j, :, :])
                    for k0 in range(0, K, 512):
                        kw = min(512, K - k0)
                        ps = ppool.tile([OUT, kw], dt)
                        nc.tensor.matmul(
                            ps[:],
                            w_t[:],
                            x_t[:, jj * K + k0 : jj * K + k0 + kw],
                            start=True,
                            stop=True,
                        )
                        nc.vector.tensor_copy(
                            out=o_t[:, jj * K + k0 : jj * K + k0 + kw], in_=ps[:]
                        )
                nc.sync.dma_start(
                    out=out_d[:, g * GCOL : (g + 1) * GCOL], in_=o_t[:]
                )
    nc.compile()
    return nc


def kernel(x, weight, context):
    global LAST_RESULT, LAST_NC
    from concourse import bass_utils

    x = np.asarray(x)
    weight = np.asarray(weight)
    context = np.asarray(context)

    B, IN = x.shape
    E, _, OUT = weight.shape
    M = _CORES
    EPC = E // M

    ctxv = context.reshape(-1).astype(np.int64)
    counts = np.bincount(ctxv, minlength=E)
    K = int(counts.max())
    K = max(8, ((K + 7) // 8) * 8)
    # group multiple experts per x/out DMA so each transfer is >=2KB/partition
    GRP = 1
    while GRP * K < 512 and EPC % (GRP * 2) == 0:
        GRP *= 2

    # stable order of samples grouped by expert
    order = np.argsort(ctxv, kind="stable")
    starts = np.zeros(E + 1, np.int64)
    starts[1:] = np.cumsum(counts)
    # for each sample (in sorted order): owning core and destination column
    e_sorted = ctxv[order]
    rank = np.arange(B, dtype=np.int64) - np.repeat(starts[:-1], counts)
    core_s = e_sorted // EPC
    col_s = (e_sorted % EPC) * K + rank

    NCOL = EPC * K
    xT = np.zeros((M, IN, NCOL), dtype=np.float32)
    xT[core_s, :, col_s] = x[order].astype(np.float32, copy=False)
    wslab = np.ascontiguousarray(
        weight.reshape(M, EPC, IN, OUT).astype(np.float32, copy=False)
    )

    nc = _build_program(IN, OUT, EPC, K, GRP)
    LAST_NC = nc
    in_maps = [{"xt": xT[c], "w": wslab[c]} for c in range(M)]
    res = bass_utils.run_bass_kernel_spmd(nc, in_maps, core_ids=list(range(M)))
    LAST_RESULT = res

    outt = np.stack([res.results[c]["outt"] for c in range(M)])  # [M, OUT, NCOL]
    out = np.empty((B, OUT), dtype=np.float32)
    out[order] = outt[core_s, :, col_s]
    return out


# revision 8
# speedup vs baseline: 2.7978x; 2.7978x over previous
"""Trainium2 Bass kernel for ContextHyperMatrix (MoE-style routed vec-mat).

Reference computation:
    w = weight[context[:, 0]]              # [B, IN, OUT] gather
    out = einsum('bx,bxy->by', x, w)       # [B, OUT]

Shapes: x [32768, 128] f32, weight [1024, 128, 128] f32, context [32768, 1] i64.

Strategy (expert-parallel, fully static SPMD device program):
  - Experts are ranked by sample count (descending); rank r maps to core
    r % 8, slot r // 8. Every core holds 128 expert slots; slot i's column
    width W[i] = max sample count over the 8 cores' rank-octet — order
    statistics across cores are tight, so sum(W) barely exceeds B/8.
  - The host routes samples: each core's x shard is x.T columns grouped by
    slot at static offsets (cumsum of W), zero-padded to W[i] per slot.
    The per-core weight slab is the core's 128 experts in slot order, so the
    device reads weights with plain sequential strided DMAs — no indirection.
  - Device per slot: matmul psum[:, off:off+W] = W_slot.T-stationary @ x.T
    columns (psum accumulates several slots, <=512 cols per PSUM bank), one
    DVE copy per bank to SBUF, chunked DMAs in/out.
  - Host scatters out.T columns back to the original sample order.

The slot widths are data-dependent *compile-time constants*: kernel() builds
and compiles the program for the observed routing each call (one program for
all 8 cores; only data differs per core).
"""

import numpy as np

# Populated by kernel() after each run; test harness reads timing from here.
LAST_RESULT = None
LAST_NC = None

_CORES = 8
_WB = 8  # expert slots per weight DMA
_PSUM_COLS = 512  # max f32 columns per PSUM bank
_CHUNK_COLS = 2048  # target columns per x/out DMA


def _plan(W):
    """Static schedule from slot widths.

    Returns (pieces, pgroups, chunks):
      pieces: per matmul: (slot, k0, kw, pg_idx, pg_off)
      pgroups: per PSUM bank: (width, chunk_idx)
      chunks: per x/out DMA: (col_lo, col_hi)
    """
    col = np.zeros(len(W) + 1, dtype=np.int64)
    col[1:] = np.cumsum(W)

    pieces = []
    pgroups = []  # [width]
    cur_w = 0
    for s, w in enumerate(W):
        k0 = 0
        while k0 < w:
            kw = min(_PSUM_COLS, w - k0)
            if cur_w + kw > _PSUM_COLS:
                pgroups.append(cur_w)
                cur_w = 0
            pieces.append((s, k0, kw, len(pgroups), cur_w))
            cur_w += kw
            k0 += kw
    if cur_w:
        pgroups.append(cur_w)

    # chunks = consecutive pgroups, sized around _CHUNK_COLS
    chunks = []
    pg_chunk = []
    lo = 0
    acc = 0
    for gi, gw in enumerate(pgroups):
        if acc and acc + gw > _CHUNK_COLS:
            chunks.append((lo, lo + acc))
            lo += acc
            acc = 0
        pg_chunk.append(len(chunks))
        acc += gw
    if acc:
        chunks.append((lo, lo + acc))

    pgroups = [(gw, pg_chunk[gi]) for gi, gw in enumerate(pgroups)]
    return col, pieces, pgroups, chunks


def _build_program(IN, OUT, W):
    import concourse.mybir as mybir
    import concourse.tile as tile
    from concourse import bacc

    EPC = len(W)
    col, pieces, pgroups, chunks = _plan(W)
    NCOL = int(col[-1])

    nc = bacc.Bacc(
        "TRN2",
        target_bir_lowering=False,
        debug=False,
        num_devices=_CORES,
    )
    dt = mybir.dt.float32
    xt_d = nc.dram_tensor("xt", [IN, NCOL], dt, kind="ExternalInput").ap()
    w_d = nc.dram_tensor("w", [EPC, IN, OUT], dt, kind="ExternalInput").ap()
    out_d = nc.dram_tensor("outt", [OUT, NCOL], dt, kind="ExternalOutput").ap()

    with tile.TileContext(nc) as tc:
        with (
            tc.tile_pool(name="xbuf", bufs=3) as xpool,
            tc.tile_pool(name="obuf", bufs=3) as opool,
            tc.tile_pool(name="wbuf", bufs=4) as wpool,
            tc.tile_pool(name="psum", bufs=6, space="PSUM") as ppool,
        ):
            x_tiles = {}
            o_tiles = {}
            for ci, (lo, hi) in enumerate(chunks):
                x_t = xpool.tile([IN, hi - lo], dt, tag="xbuf", name=f"x_t{ci}")
                nc.sync.dma_start(out=x_t[:], in_=xt_d[:, lo:hi])
                x_tiles[ci] = (x_t, lo)
                o_tiles[ci] = (opool.tile([OUT, hi - lo], dt, tag="obuf", name=f"o_t{ci}"), lo)

            w_tiles = {}
            ps_tiles = {}
            pg_done = {}
            pg_off = {}
            acc = 0
            for gi, (gw, ci) in enumerate(pgroups):
                pg_off[gi] = acc
                acc += gw

            for s, k0, kw, gi, po in pieces:
                b = s // _WB
                if b not in w_tiles:
                    j0 = b * _WB
                    w_t = wpool.tile([IN, _WB, OUT], dt, tag="wbuf", name=f"w_t{b}")
                    nc.sync.dma_start(
                        out=w_t[:],
                        in_=w_d[j0 : j0 + _WB, :, :].transpose([1, 0, 2]),
                    )
                    w_tiles[b] = w_t
                if gi not in ps_tiles:
                    ps_tiles[gi] = ppool.tile([OUT, pgroups[gi][0]], dt, tag="psum", name=f"ps{gi}")
                ps = ps_tiles[gi]
                ci = pgroups[gi][1]
                x_t, xlo = x_tiles[ci]
                xoff = int(col[s]) + k0 - xlo
                nc.tensor.matmul(
                    ps[:, po : po + kw],
                    w_tiles[b][:, s - b * _WB, :],
                    x_t[:, xoff : xoff + kw],
                    start=True,
                    stop=True,
                )
                pg_done.setdefault(gi, 0)
                pg_done[gi] += kw
                if pg_done[gi] == pgroups[gi][0]:
                    o_t, olo = o_tiles[ci]
                    ooff = pg_off[gi] - olo
                    nc.vector.tensor_copy(
                        out=o_t[:, ooff : ooff + pgroups[gi][0]], in_=ps[:]
                    )

            for ci, (lo, hi) in enumerate(chunks):
                o_t, _ = o_tiles[ci]
                nc.sync.dma_start(out=out_d[:, lo:hi], in_=o_t[:])
    nc.compile()
    return nc


def kernel(x, weight, context):
    global LAST_RESULT, LAST_NC
    from concourse import bass_utils

    x = np.asarray(x)
    weight = np.asarray(weight)
    context = np.asarray(context)

    B, IN = x.shape
    E, _, OUT = weight.shape
    M = _CORES
    EPC = E // M

    ctxv = context.reshape(-1).astype(np.int64)
    counts = np.bincount(ctxv, minlength=E)

    # rank experts by count desc; rank r -> core r % M, slot r // M
    ranked = np.argsort(-counts, kind="stable")
    inv_rank = np.empty(E, dtype=np.int64)
    inv_rank[ranked] = np.arange(E)
    # slot widths: max count within each rank-octet (= first of octet)
    W = np.maximum(counts[ranked].reshape(EPC, M).max(axis=1), 1).astype(np.int64)
    col = np.zeros(EPC + 1, dtype=np.int64)
    col[1:] = np.cumsum(W)
    NCOL = int(col[-1])

    # sample -> (core, column)
    order = np.argsort(ctxv, kind="stable")
    starts = np.zeros(E + 1, np.int64)
    starts[1:] = np.cumsum(counts)
    e_sorted = ctxv[order]
    rank_within = np.arange(B, dtype=np.int64) - np.repeat(starts[:-1], counts)
    r_sorted = inv_rank[e_sorted]
    core_s = r_sorted % M
    col_s = col[r_sorted // M] + rank_within

    xT = np.zeros((M, IN, NCOL), dtype=np.float32)
    xT[core_s, :, col_s] = x[order].astype(np.float32, copy=False)
    # per-core weight slab in slot order: w_slab[c][i] = weight[ranked[i*M+c]]
    w_slab = np.ascontiguousarray(
        weight[ranked.reshape(EPC, M)].transpose(1, 0, 2, 3).astype(
            np.float32, copy=False
        )
    )

    nc = _build_program(IN, OUT, list(W))
    LAST_NC = nc
    in_maps = [{"xt": xT[c], "w": w_slab[c]} for c in range(M)]
    res = bass_utils.run_bass_kernel_spmd(nc, in_maps, core_ids=list(range(M)))
    LAST_RESULT = res

    outt = np.stack([res.results[c]["outt"] for c in range(M)])  # [M, OUT, NCOL]
    out = np.empty((B, OUT), dtype=np.float32)
    out[order] = outt[core_s, :, col_s]
    return out
